# revision 1
# baseline (speedup 1.0000x reference)
"""Trainium2 Bass kernel for nn_LocalInteractionsLayer.

Reference computation:
    seq_pairs [B=16, C=8, L=4096, 2] f32
    top = seq_pairs[..., 0]; bot = seq_pairs[..., 1]
    out[b, p, c*225 + i*15 + j] = top[b, c, p+i] * bot[b, c, p+j]
    for p in [0, P), i,j in [0, 15), P = L - 14 = 4082
    -> out [16, 4082, 1800] f32 (~470 MB; heavily output-write bound).

Strategy:
  - Data-parallel over batch: 2 batches per core on 8 cores.
  - Host pre-builds the 15-wide sliding windows (a 15x data expansion of the
    tiny 4 MB input) laid out so each SBUF partition p holds the windows for
    output position t*128+p contiguously. One fully-contiguous DMA load per
    8-tile group brings in both top and bot windows.
  - On device, a single vector-engine tensor_mul per 128-position tile
    computes the whole [128, 8, 15, 15] outer-product block using broadcast
    (step-0) access patterns. The output tile [128, 1800] is stored with one
    fully-contiguous ~921 KB DMA per tile (64 multiplies + 64 stores per
    core). Measured ~199 us/core, ~1.06x the DMA-roofline cost model.
"""

import sys

if "/opt/trn_rl_repo" not in sys.path:
    sys.path.insert(0, "/opt/trn_rl_repo")

import numpy as np
from numpy.lib.stride_tricks import sliding_window_view

import concourse.tile as tile
from concourse import bacc, mybir
from concourse.bass_utils import run_bass_kernel_spmd

W = 15            # window length (2*7+1)
WPAD = W - 1
B, C, L = 16, 8, 4096
P = L - WPAD      # 4082 valid output positions
FREE = C * W * W  # 1800
NCORES = 8
BPC = B // NCORES  # batches per core = 2
NT = L // 128      # 32 position-tiles per batch (last one partially valid)
NG = 4             # tile groups per batch (DMA load batching)
GT = NT // NG      # 8 tiles per group
GW = GT * C * W    # free size of one operand group = 960

_BUILD_CACHE: dict = {}


def _build(loop_iters: int = 1, load_eng: str = "scalar", store_mode: str = "sync",
           in_bufs: int = 3, out_bufs: int = 4):
    """Build + compile the per-core Bacc program (identical on all 8 cores)."""
    nc = bacc.Bacc("TRN2", target_bir_lowering=False, debug=False, num_devices=NCORES)
    dt = mybir.dt.float32

    # inw[b, g, :, 0:GW] = top windows, [.., GW:2*GW] = bot windows
    inw_d = nc.dram_tensor("inw", [BPC, NG, 128, 2 * GW], dt, kind="ExternalInput")
    out_d = nc.dram_tensor("out", [BPC, P, FREE], dt, kind="ExternalOutput")

    with tile.TileContext(nc) as tc:
        with (
            tc.tile_pool(name="inp", bufs=in_bufs) as inp,
            tc.tile_pool(name="outp", bufs=out_bufs) as outp,
        ):
            def _body(_it=None):
                for b in range(BPC):
                    for g in range(NG):
                        inwt = inp.tile([128, 2 * GW], dt, tag="inw")
                        # Loads ride the ACT HWDGE ring so they never queue
                        # behind ~1MB output stores on the SP ring.
                        {"scalar": nc.scalar, "sync": nc.sync,
                         "gpsimd": nc.gpsimd}[load_eng].dma_start(
                            inwt[:], inw_d[b, g])
                        for tq in range(GT):
                            t = g * GT + tq
                            ot = outp.tile([128, FREE], dt, tag="ot")
                            a_src = inwt[:, tq * C * W : (tq + 1) * C * W]
                            b_src = inwt[:, GW + tq * C * W : GW + (tq + 1) * C * W]
                            a = (
                                a_src.rearrange("p (c i) -> p c i", c=C)
                                .unsqueeze(3)
                                .broadcast_to((128, C, W, W))
                            )
                            bb = (
                                b_src.rearrange("p (c j) -> p c j", c=C)
                                .unsqueeze(2)
                                .broadcast_to((128, C, W, W))
                            )
                            o = ot[:].rearrange("p (c i j) -> p c i j", c=C, i=W)
                            nc.vector.tensor_mul(o, a, bb)
                            rows = min(128, P - t * 128)
                            # Alternate stores across the two HWDGE rings
                            # (SP / ACT) for descriptor-generation parallelism.
                            if store_mode == "alt":
                                st_eng = nc.sync if t % 2 == 0 else nc.scalar
                            else:
                                st_eng = nc.sync
                            st_eng.dma_start(
                                out_d[b, t * 128 : t * 128 + rows, :], ot[:rows, :]
                            )

            if loop_iters == 1:
                _body()
            else:
                with tc.For_i(0, loop_iters, 1) as it:
                    _body(it)
    nc.compile()
    return nc


def _get_built(loop_iters: int = 1):
    nc = _BUILD_CACHE.get(loop_iters)
    if nc is None:
        nc = _build(loop_iters)
        _BUILD_CACHE[loop_iters] = nc
    return nc


def _prep(seq_pairs: np.ndarray) -> np.ndarray:
    """Host-side window expansion into the DMA-friendly device layout.

    inw[b, g, p, s*GW + tq*C*W + c*W + i] = seq_pairs[b, c, (g*GT+tq)*128 + p + i, s]
    (positions past P-1 read zero padding; those rows are never stored).
    """
    sp = np.ascontiguousarray(seq_pairs, dtype=np.float32)
    padded = np.zeros((B, C, L + WPAD, 2), np.float32)
    padded[:, :, :L] = sp
    win = sliding_window_view(padded, W, axis=2)  # [B, C, L, 2, W]
    v = win.reshape(B, C, NG, GT, 128, 2, W)
    v = np.ascontiguousarray(v.transpose(0, 2, 4, 5, 3, 1, 6))  # [b,g,p,s,tq,c,i]
    return v.reshape(B, NG, 128, 2 * GW)


def kernel(seq_pairs: np.ndarray) -> np.ndarray:
    assert tuple(np.shape(seq_pairs)) == (B, C, L, 2), (
        f"expected seq_pairs shape {(B, C, L, 2)}, got {np.shape(seq_pairs)}"
    )
    inw = _prep(seq_pairs)
    nc = _get_built()
    in_maps = [{"inw": inw[k * BPC : (k + 1) * BPC]} for k in range(NCORES)]
    last_err = None
    for _attempt in range(3):
        try:
            res = run_bass_kernel_spmd(nc, in_maps, list(range(NCORES))).results
            break
        except Exception as err:  # transient axon/PJRT hiccups — retry
            last_err = err
    else:
        raise last_err
    return np.concatenate([res[k]["out"] for k in range(NCORES)], axis=0)



# revision 15
# speedup vs baseline: 1.2207x; 1.2207x over previous
"""Trainium2 Bass kernel for nn_LocalInteractionsLayer.

Reference computation:
    seq_pairs [B=16, C=8, L=4096, 2] f32
    top = seq_pairs[..., 0]; bot = seq_pairs[..., 1]
    out[b, p, c*225 + i*15 + j] = top[b, c, p+i] * bot[b, c, p+j]
    for p in [0, P), i,j in [0, 15), P = L - 14 = 4082
    -> out [16, 4082, 1800] f32 (~470 MB; heavily output-write bound).

Strategy:
  - Data-parallel over batch: 2 batches per core on 8 cores.
  - All device I/O in float16 (the 2e-2 rel-err budget dwarfs f16's ~5e-4):
    halves both the window loads and the dominant output-store traffic vs
    f32, cutting the per-core DMA roofline from ~185 us to ~93 us.
  - Host pre-builds the 15-wide sliding windows (a 15x expansion of the tiny
    4 MB input) laid out so each SBUF partition p holds the windows for
    output position t*128+p contiguously; one contiguous DMA load per
    8-tile group brings in both top and bot windows (f16).
  - On device, the [128, C, W, W] outer-product blocks are computed two
    position-tiles per instruction with broadcast (step-0) access patterns,
    split between the vector (DVE, ~2.0 us/tile-pair... 3.9 us/pair) and
    gpsimd (Pool, ~7.3 us/pair) engines in a ~21:11 ratio so both finish in
    ~82 us, hidden under the DMA stream. Stores are one ~450 KB contiguous
    DMA per position-tile on the SP ring; loads ride the ACT ring.
"""

import sys

if "/opt/trn_rl_repo" not in sys.path:
    sys.path.insert(0, "/opt/trn_rl_repo")

import numpy as np
from numpy.lib.stride_tricks import sliding_window_view

import concourse.tile as tile
from concourse import bacc, mybir
from concourse.bass_utils import run_bass_kernel_spmd

W = 15            # window length (2*7+1)
WPAD = W - 1
B, C, L = 16, 8, 4096
P = L - WPAD      # 4082 valid output positions
FREE = C * W * W  # 1800
NCORES = 8
BPC = B // NCORES  # batches per core = 2
NT = L // 128      # 32 position-tiles per batch (last one partially valid)
NG = 4             # tile groups per batch (DMA load batching)
GT = NT // NG      # 8 tiles per group
GW = GT * C * W    # free size of one operand group = 960
CW = C * W         # 120

_BUILD_CACHE: dict = {}


def _build(loop_iters: int = 1, in_bufs: int = 3, out_bufs: int = 6,
           first_fast: bool = True):
    """Build + compile the per-core Bacc program (identical on all 8 cores).

    The work is issued in "chunks" of 1 or 2 position-tiles. Each chunk is one
    tensor_mul on either the DVE or the Pool engine (split ~42:22 tiles so both
    finish together, hidden under the DMA stream) followed by one output-store
    DMA per tile. With first_fast, tile 0 gets a dedicated small operand load
    and a single-tile multiply so the first store enters the DMA pipeline ~3 us
    earlier (total time ~= ramp + serialized-DMA time).
    """
    nc = bacc.Bacc("TRN2", target_bir_lowering=False, debug=False, num_devices=NCORES)
    dt = mybir.dt.float16

    # inw[b, g, :, 0:GW] = top windows, [.., GW:2*GW] = bot windows (f16)
    inw_d = nc.dram_tensor("inw", [BPC, NG, 128, 2 * GW], dt, kind="ExternalInput")
    out_d = nc.dram_tensor("out", [BPC, P, FREE], dt, kind="ExternalOutput")

    with tile.TileContext(nc) as tc:
        with (
            tc.tile_pool(name="inp", bufs=in_bufs) as inp,
            tc.tile_pool(name="outp", bufs=out_bufs) as outp,
        ):
            def _body(_it=None):
                # Engine split: DVE ~2030 ns/tile vs Pool ~3728 ns/tile.
                # Pool takes 11 of the 31-32 tile-pairs (evenly interleaved,
                # Bresenham) so both engines finish in ~82 us, hidden under
                # the ~93 us DMA stream. Singles stay on DVE.
                state = {"acc": 0, "npairs": 31 if first_fast else 32}

                def pick_engine(n_tiles, force_dve=False):
                    if force_dve or n_tiles == 1:
                        return nc.vector
                    state["acc"] += 11
                    if state["acc"] >= state["npairs"]:
                        state["acc"] -= state["npairs"]
                        return nc.gpsimd
                    return nc.vector

                def compute_and_store(inwt, base_tq_off, bot_off, b, t0,
                                      n_tiles, force_dve=False):
                    ot = outp.tile([128, n_tiles * FREE], dt, tag="ot")
                    a_src = inwt[:, base_tq_off : base_tq_off + n_tiles * CW]
                    b_src = inwt[
                        :,
                        bot_off + base_tq_off : bot_off + base_tq_off
                        + n_tiles * CW,
                    ]
                    a = (
                        a_src.rearrange("p (t c i) -> p t c i", t=n_tiles, c=C)
                        .unsqueeze(4)
                        .broadcast_to((128, n_tiles, C, W, W))
                    )
                    bb = (
                        b_src.rearrange("p (t c j) -> p t c j", t=n_tiles, c=C)
                        .unsqueeze(3)
                        .broadcast_to((128, n_tiles, C, W, W))
                    )
                    o = ot[:].rearrange(
                        "p (t c i j) -> p t c i j", t=n_tiles, c=C, i=W
                    )
                    pick_engine(n_tiles, force_dve).tensor_mul(o, a, bb)
                    for h in range(n_tiles):
                        rows = min(128, P - (t0 + h) * 128)
                        nc.sync.dma_start(
                            out_d[b, (t0 + h) * 128 : (t0 + h) * 128 + rows, :],
                            ot[:rows, h * FREE : h * FREE + FREE],
                        )

                for b in range(BPC):
                    for g in range(NG):
                        starter = first_fast and b == 0 and g == 0
                        if starter:
                            # Small dedicated load of tile 0's operands so the
                            # first store enters the DMA stream early.
                            inwt0 = inp.tile([128, 2 * CW], dt, tag="inw0")
                            src0 = inw_d[0, 0].rearrange(
                                "p (s q) -> p s q", s=2
                            )[:, :, 0:CW]
                            dst0 = inwt0[:].rearrange("p (s q) -> p s q", s=2)
                            nc.scalar.dma_start(dst0, src0)
                            compute_and_store(inwt0, 0, CW, 0, 0, 1,
                                              force_dve=True)
                        inwt = inp.tile([128, 2 * GW], dt, tag="inw")
                        # Loads ride the ACT HWDGE ring so they never queue
                        # behind the big output stores on the SP ring.
                        nc.scalar.dma_start(inwt[:], inw_d[b, g])
                        if starter:
                            chunks = [(1, 1), (2, 2), (4, 2), (6, 2)]
                        else:
                            chunks = [(0, 2), (2, 2), (4, 2), (6, 2)]
                        for tq, n_tiles in chunks:
                            compute_and_store(
                                inwt, tq * CW, GW, b, g * GT + tq, n_tiles
                            )

            if loop_iters == 1:
                _body()
            else:
                with tc.For_i(0, loop_iters, 1) as it:
                    _body(it)
    nc.compile()
    return nc


def _get_built(loop_iters: int = 1):
    nc = _BUILD_CACHE.get(loop_iters)
    if nc is None:
        nc = _build(loop_iters)
        _BUILD_CACHE[loop_iters] = nc
    return nc


def _prep(seq_pairs: np.ndarray) -> np.ndarray:
    """Host-side window expansion into the DMA-friendly device layout (f16).

    inw[b, g, p, s*GW + tq*C*W + c*W + i] = seq_pairs[b, c, (g*GT+tq)*128 + p + i, s]
    (positions past P-1 read zero padding; those rows are never stored).
    """
    sp = np.ascontiguousarray(seq_pairs, dtype=np.float32)
    padded = np.zeros((B, C, L + WPAD, 2), np.float32)
    padded[:, :, :L] = sp
    win = sliding_window_view(padded, W, axis=2)  # [B, C, L, 2, W]
    v = win.reshape(B, C, NG, GT, 128, 2, W)
    v = v.transpose(0, 2, 4, 5, 3, 1, 6)  # [b,g,p,s,tq,c,i]
    return np.ascontiguousarray(v, dtype=np.float16).reshape(B, NG, 128, 2 * GW)


def kernel(seq_pairs: np.ndarray) -> np.ndarray:
    assert tuple(np.shape(seq_pairs)) == (B, C, L, 2), (
        f"expected seq_pairs shape {(B, C, L, 2)}, got {np.shape(seq_pairs)}"
    )
    inw = _prep(seq_pairs)
    nc = _get_built()
    in_maps = [{"inw": inw[k * BPC : (k + 1) * BPC]} for k in range(NCORES)]
    last_err = None
    for _attempt in range(3):
        try:
            res = run_bass_kernel_spmd(nc, in_maps, list(range(NCORES))).results
            break
        except Exception as err:  # transient axon/PJRT hiccups — retry
            last_err = err
    else:
        raise last_err
    out = np.concatenate([res[k]["out"] for k in range(NCORES)], axis=0)
    return np.ascontiguousarray(out.astype(np.float32))


# revision 16
# speedup vs baseline: 2.2164x; 1.8156x over previous
"""Trainium2 Bass kernel for nn_LocalInteractionsLayer.

Reference computation:
    seq_pairs [B=16, C=8, L=4096, 2] f32
    top = seq_pairs[..., 0]; bot = seq_pairs[..., 1]
    out[b, p, c*225 + i*15 + j] = top[b, c, p+i] * bot[b, c, p+j]
    for p in [0, P), i,j in [0, 15), P = L - 14 = 4082
    -> out [16, 4082, 1800] f32 (~470 MB; heavily output-write bound).

Strategy (v3, row-packed):
  - Data-parallel over batch: 2 batches per core on 8 cores.
  - All device I/O in float16 (2e-2 rel-err budget dwarfs f16's ~4e-4),
    halving the dominant output-store traffic vs f32.
  - Row packing: SBUF partition p of a position-tile holds TWO adjacent
    output rows (2p, 2p+1), so every store descriptor covers 7200
    contiguous bytes of DRAM. Measured on HW: 7200B descriptors sustain
    ~334 GB/s vs only ~228 GB/s for the 3600B descriptors a plain f16
    row-per-partition layout produces (and 14400B descs are slow again).
  - 16-wide host-prebuilt windows: rows 2p and 2p+1 share one 16-value
    window per (channel, operand), so the host-side expansion is 8x
    instead of 15x (2.1 MB of loads per core instead of 3.9 MB).
  - Compute per tile (256 positions, [128, 2*1800] f16):
      * row 2p's full 15x15 outer block: one DVE tensor_mul (1800/part)
      * row 2p+1 reuses row 2p's block shifted by (1,1): the shared 14x14
        interior is a scalar-engine (ACT) copy, only the new L-shape
        (i=14 row, j=14 column) is computed by two small DVE muls.
    => DVE ~83 us, ACT ~54 us, both hidden under the ~94 us DMA stream.
  - Stores ride the SP HWDGE ring (one 900 KB DMA per tile), loads ride
    the ACT ring, 2 group-loads per batch.
"""

import sys

if "/opt/trn_rl_repo" not in sys.path:
    sys.path.insert(0, "/opt/trn_rl_repo")

import numpy as np
from numpy.lib.stride_tricks import sliding_window_view

import concourse.tile as tile
from concourse import bacc, mybir
from concourse.bass_utils import run_bass_kernel_spmd

W = 15            # window length (2*7+1)
WPAD = W - 1
U = 16            # shared window width for a row pair (W + 1)
B, C, L = 16, 8, 4096
P = L - WPAD      # 4082 valid output positions
BLK = W * W       # 225
FREE = C * BLK    # 1800
NCORES = 8
BPC = B // NCORES  # batches per core = 2
RP = 2             # output rows packed per SBUF partition
TPOS = RP * 128    # positions per tile = 256
NT = L // TPOS     # 16 position-tiles per batch (last one partial: 242 rows)
NG = 2             # tile groups per batch (DMA load batching)
GT = NT // NG      # 8 tiles per group
TW = 2 * C * U     # per-tile operand window elems per partition = 256
GW = GT * TW       # free size of one group load = 2048

_BUILD_CACHE: dict = {}


def _build(loop_iters: int = 1, in_bufs: int = 4, out_bufs: int = 6,
           first_fast: bool = True, rects_on_pool: bool = False,
           copies_on: str = "scalar"):
    """Build + compile the per-core Bacc program (identical on all 8 cores)."""
    nc = bacc.Bacc("TRN2", target_bir_lowering=False, debug=False, num_devices=NCORES)
    dt = mybir.dt.float16

    # inw[b, g, p, tq*TW + s*C*U + c*U + u] = window value u for operand s,
    # channel c, output rows (2p, 2p+1) of tile (g*GT + tq).
    inw_d = nc.dram_tensor("inw", [BPC, NG, 128, GW], dt, kind="ExternalInput")
    out_d = nc.dram_tensor("out", [BPC, P, FREE], dt, kind="ExternalOutput")

    cpeng = {"scalar": None, "vector": None, "gpsimd": None}

    with tile.TileContext(nc) as tc:
        with (
            tc.tile_pool(name="inp", bufs=in_bufs) as inp,
            tc.tile_pool(name="outp", bufs=out_bufs) as outp,
        ):
            def compute_and_store(opw, b, t):
                """opw: [128, TW] operand view (s, c, u); tile t of batch b."""
                ot = outp.tile([128, RP * FREE], dt, tag="ot")
                v = opw.rearrange("p (s c u) -> p s c u", s=2, c=C)
                o4 = ot[:].rearrange("p (r c i j) -> p r c i j", r=RP, c=C, i=W)
                # Row 2p: full 15x15 outer block, one big DVE mul.
                a0 = v[:, 0, :, 0:W].unsqueeze(3).broadcast_to((128, C, W, W))
                b0 = v[:, 1, :, 0:W].unsqueeze(2).broadcast_to((128, C, W, W))
                nc.vector.tensor_mul(o4[:, 0], a0, b0)
                # Row 2p+1, new L-shape (window positions shifted by +1):
                reng = nc.gpsimd if rects_on_pool else nc.vector
                # rect A: i = 14 row -> top u=15, bot u=1..15
                aA = v[:, 0, :, W].unsqueeze(2).broadcast_to((128, C, W))
                bA = v[:, 1, :, 1:U]
                reng.tensor_mul(o4[:, 1, :, W - 1, :], aA, bA)
                # rect B: j = 14 col, i = 0..13 -> top u=1..14, bot u=15
                aB = v[:, 0, :, 1:W]
                bB = v[:, 1, :, W].unsqueeze(2).broadcast_to((128, C, W - 1))
                reng.tensor_mul(o4[:, 1, :, 0 : W - 1, W - 1], aB, bB)
                # Row 2p+1 shared 14x14 interior = row 2p block shifted (1,1).
                ceng = {"scalar": nc.scalar, "vector": nc.vector,
                        "gpsimd": nc.gpsimd}[copies_on]
                ceng.copy(
                    o4[:, 1, :, 0 : W - 1, 0 : W - 1],
                    o4[:, 0, :, 1:W, 1:W],
                )
                # Store: 2 adjacent DRAM rows per partition -> 7200B descs.
                base = t * TPOS
                npart = min(128, (P - base) // RP)
                dst = out_d[b, base : base + RP * npart, :].rearrange(
                    "(p r) f -> p (r f)", r=RP
                )
                nc.sync.dma_start(dst, ot[:npart, :])

            def _body(_it=None):
                for b in range(BPC):
                    for g in range(NG):
                        starter = first_fast and b == 0 and g == 0
                        if starter:
                            # Tiny dedicated load of tile 0's operands so the
                            # first store enters the DMA stream early.
                            inwt0 = inp.tile([128, TW], dt, tag="inw0")
                            nc.scalar.dma_start(inwt0[:], inw_d[0, 0, :, 0:TW])
                            compute_and_store(inwt0[:], 0, 0)
                        inwt = inp.tile([128, GW], dt, tag="inw")
                        nc.scalar.dma_start(inwt[:], inw_d[b, g])
                        for tq in range(1 if starter else 0, GT):
                            compute_and_store(
                                inwt[:, tq * TW : (tq + 1) * TW],
                                b, g * GT + tq,
                            )

            if loop_iters == 1:
                _body()
            else:
                with tc.For_i(0, loop_iters, 1) as it:
                    _body(it)
    nc.compile()
    return nc


def _get_built(loop_iters: int = 1):
    nc = _BUILD_CACHE.get(loop_iters)
    if nc is None:
        nc = _build(loop_iters)
        _BUILD_CACHE[loop_iters] = nc
    return nc


def _prep(seq_pairs: np.ndarray) -> np.ndarray:
    """Host-side 16-wide window expansion into the device layout (f16).

    inw[b, g, p, ((tq*2 + s)*C + c)*U + u]
        = seq_pairs[b, c, (g*GT+tq)*256 + 2p + u, s]
    (positions past L-1 read zero padding; rows past P-1 are never stored).
    """
    sp = np.ascontiguousarray(seq_pairs, dtype=np.float32)
    padded = np.zeros((B, C, L + WPAD, 2), np.float32)
    padded[:, :, :L] = sp
    win16 = sliding_window_view(padded, U, axis=2)  # [B, C, 4095, 2, U]
    ev = win16[:, :, 0 : NT * TPOS : RP]            # [B, C, 2048, 2, U]
    v = ev.reshape(B, C, NG, GT, 128, 2, U)
    v = v.transpose(0, 2, 4, 3, 5, 1, 6)            # [b, g, p, tq, s, c, u]
    return np.ascontiguousarray(v, dtype=np.float16).reshape(B, NG, 128, GW)


def kernel(seq_pairs: np.ndarray) -> np.ndarray:
    assert tuple(np.shape(seq_pairs)) == (B, C, L, 2), (
        f"expected seq_pairs shape {(B, C, L, 2)}, got {np.shape(seq_pairs)}"
    )
    inw = _prep(seq_pairs)
    nc = _get_built()
    in_maps = [{"inw": inw[k * BPC : (k + 1) * BPC]} for k in range(NCORES)]
    last_err = None
    for _attempt in range(3):
        try:
            res = run_bass_kernel_spmd(nc, in_maps, list(range(NCORES))).results
            break
        except Exception as err:  # transient axon/PJRT hiccups — retry
            last_err = err
    else:
        raise last_err
    out = np.concatenate([res[k]["out"] for k in range(NCORES)], axis=0)
    return np.ascontiguousarray(out.astype(np.float32))
